# revision 35
# baseline (speedup 1.0000x reference)
"""Multi-head attention (16 heads, RoPE, causal) for Trainium2, 8 NeuronCores.

Sharding: data-parallel over batch (2) x tensor-parallel over head groups (4),
one (batch, head-group-of-4) pair per core. Each core computes its 4 heads'
attention feature-major (transposed) and a partial output projection
outT = Wo_slice^T @ Y^T [1024, 2048] in bf16; the host sums the 4 partials
per batch in f32 and transposes back.

v2: single fused software pipeline over 512-wide l-chunks. Causality means
attention chunk c only consumes K/V chunks <= c, so projections+RoPE,
attention (S -> exp -> O), and the output projection all stream together
instead of running as serial phases:
  - S^T matmuls for the two heads of an nt-tile are emitted back-to-back with
    stationary/moving operands at base partitions 0 and 64, auto-deriving PE
    tile_position (0,0)/(64,0) -> both K=64 matmuls run concurrently in the
    128x128 array (2x effective S throughput).
  - exp runs on [128, 1024] groups (head-pair x one j-tile), keeping the
    scalar queue almost pure exp; softmax denominator comes from a
    ones-column appended to V (O_aug row 64), so no partition reductions.
  - causal masking of diagonal j-tiles is one wide bf16 multiply against a
    sliding window of a cyclic [128, 1536] host-built tri mask.
  - next chunk's projection matmuls and the previous chunk's Wo matmuls are
    interleaved as "pieces" between attention groups (matmuls at group g,
    psum evacuation at group g+1) so tensor/scalar/vector never block.
  - exp has no max-subtraction: logits are tiny for this problem family; a
    host-side spectral bound checks this and falls back to numpy otherwise.
"""

import sys

sys.path.insert(0, "/opt/trn_rl_repo")
sys.path.insert(0, "/root/.axon_site")

import numpy as np

B, L, D = 2, 2048, 1024
H = 16                  # total heads
HD = 64                 # head dim
HPC = 4                 # heads per core
NCORES = 8
NT = 2                  # 128-row tiles per core of Q^T/K^T/Y^T (HPC*HD/128)
LC = L // 512           # 512-wide l chunks
KC = D // 128           # 128-deep contraction chunks over model dim
LT = L // 128           # 128-row l tiles

_cache = {}


def _build_nc(causal: bool):
    import contextlib

    import concourse.bass as bass
    import concourse.tile as tile
    from concourse import bacc, mybir

    F32 = mybir.dt.float32
    BF16 = mybir.dt.bfloat16
    EXP = mybir.ActivationFunctionType.Exp

    nc = bacc.Bacc("TRN2", target_bir_lowering=False, debug=False, num_devices=NCORES)

    xT = nc.dram_tensor("xT", [D, L], BF16, kind="ExternalInput")
    wq = nc.dram_tensor("wq", [D, 256], BF16, kind="ExternalInput")
    wk = nc.dram_tensor("wk", [D, 256], BF16, kind="ExternalInput")
    wv = nc.dram_tensor("wv", [D, 256], BF16, kind="ExternalInput")
    wo = nc.dram_tensor("wo", [256, D], BF16, kind="ExternalInput")
    cos128 = nc.dram_tensor("cos128", [128, L], BF16, kind="ExternalInput")
    srot128 = nc.dram_tensor("srot128", [128, L], BF16, kind="ExternalInput")
    mkq = nc.dram_tensor("mkq", [128, 1536], BF16, kind="ExternalInput")
    outT = nc.dram_tensor("outT", [D, L], BF16, kind="ExternalOutput")

    with tile.TileContext(nc) as tc, \
         nc.allow_low_precision(reason="bf16 matmul pipeline by design"), \
         contextlib.ExitStack() as ctx:
        p_w = ctx.enter_context(tc.tile_pool(name="p_w", bufs=1))
        p_wo = ctx.enter_context(tc.tile_pool(name="p_wo", bufs=1))
        p_const = ctx.enter_context(tc.tile_pool(name="p_const", bufs=1))
        p_x = ctx.enter_context(tc.tile_pool(name="p_x", bufs=4))
        p_qt = ctx.enter_context(tc.tile_pool(name="p_qt", bufs=2))
        p_kt = ctx.enter_context(tc.tile_pool(name="p_kt", bufs=2))
        p_yt = ctx.enter_context(tc.tile_pool(name="p_yt", bufs=2))
        p_v = ctx.enter_context(tc.tile_pool(name="p_v", bufs=16))
        p_pt = ctx.enter_context(tc.tile_pool(name="p_pt", bufs=4))
        p_ev = ctx.enter_context(tc.tile_pool(name="p_ev", bufs=2))
        p_oc = ctx.enter_context(tc.tile_pool(name="p_oc", bufs=4))
        p_z = ctx.enter_context(tc.tile_pool(name="p_z", bufs=2))
        p_mm = ctx.enter_context(tc.tile_pool(name="p_mm", bufs=2, space="PSUM"))
        p_st = ctx.enter_context(tc.tile_pool(name="p_st", bufs=2, space="PSUM"))
        p_oa = ctx.enter_context(tc.tile_pool(name="p_oa", bufs=2, space="PSUM"))

        # ---- engine warmup during the startup barrier / first DMAs ------
        # ~18 dummy matmuls warm the PE HAM clock-gate to 8/8; a dummy exp
        # pre-loads the ACT table (~2.7us); a dummy partition_broadcast
        # pre-loads the gpsimd ucode (~6us first-call cost)
        warm = p_ev.tile([128, 512], BF16, tag="qraw", name="warm")
        nc.gpsimd.memset(warm[:, :], 0.0)
        warmz = p_z.tile([1, 512], F32, tag="zs", name="warmz")
        nc.gpsimd.memset(warmz[:, :], 1.0)
        wpss = [p_mm.tile([128, 512], F32, tag="mm", name=f"wps{i}")
                for i in range(2)]
        for i in range(9):
            nc.tensor.matmul(wpss[i % 2][:, :], warm[:, 0:128],
                             warm[:, :], start=True, stop=True)
        wexp = p_pt.tile([128, 1024], BF16, tag="pt", name="wexp")
        nc.scalar.activation(wexp[:, 0:64], warm[:, 0:64], EXP)
        wzb = p_z.tile([64, 512], F32, tag="zb", name="wzb")
        nc.gpsimd.partition_broadcast(wzb[:, :], warmz[0:1, :])

        # ---- input DMAs: one batched transfer per tensor, ordered so the
        # first projection's operands land first --------------------------
        x_sb = {}

        def dma_x(lc, halves=1):
            x_t = p_x.tile([128, KC, 512], BF16, tag="x", name=f"x{lc}")
            src = xT.ap().rearrange("(kc p) l -> p kc l", kc=KC)
            step = KC // halves
            for h in range(halves):
                nc.sync.dma_start(
                    out=x_t[:, h * step:(h + 1) * step, :],
                    in_=src[:, h * step:(h + 1) * step,
                            lc * 512:(lc + 1) * 512])
            x_sb[lc] = x_t

        def dma_w(src, name, halves=1):
            w_t = p_w.tile([128, KC, 256], BF16, tag=name, name=name)
            sap = src.ap().rearrange("(kc p) c -> p kc c", kc=KC)
            step = KC // halves
            for h in range(halves):
                nc.sync.dma_start(out=w_t[:, h * step:(h + 1) * step, :],
                                  in_=sap[:, h * step:(h + 1) * step, :])
            return w_t

        # interleave the first weight/x transfers so Q's low-kc matmuls can
        # begin before the full tensors land
        wq_t = p_w.tile([128, KC, 256], BF16, tag="wq", name="wq")
        x0_t = p_x.tile([128, KC, 512], BF16, tag="x", name="x0")
        wq_src = wq.ap().rearrange("(kc p) c -> p kc c", kc=KC)
        x_src = xT.ap().rearrange("(kc p) l -> p kc l", kc=KC)
        for h in range(2):
            nc.sync.dma_start(out=wq_t[:, h * 4:(h + 1) * 4, :],
                              in_=wq_src[:, h * 4:(h + 1) * 4, :])
            nc.sync.dma_start(out=x0_t[:, h * 4:(h + 1) * 4, :],
                              in_=x_src[:, h * 4:(h + 1) * 4, 0:512])
        x_sb[0] = x0_t
        wk_t = dma_w(wk, "wk")
        cos_t = p_const.tile([128, L], BF16, tag="cos")
        srot_t = p_const.tile([128, L], BF16, tag="srot")
        nc.sync.dma_start(out=cos_t[:, 0:512], in_=cos128.ap()[:, 0:512])
        nc.sync.dma_start(out=srot_t[:, 0:512], in_=srot128.ap()[:, 0:512])
        wv_t = dma_w(wv, "wv")
        dma_x(1)
        nc.sync.dma_start(out=cos_t[:, 512:], in_=cos128.ap()[:, 512:])
        nc.sync.dma_start(out=srot_t[:, 512:], in_=srot128.ap()[:, 512:])
        wo_t = p_wo.tile([128, 2, D], BF16, tag="wo")
        nc.sync.dma_start(
            out=wo_t, in_=wo.ap().rearrange("(kc p) c -> p kc c", kc=2))
        mq_t = p_const.tile([128, 1536], BF16, tag="mask")
        nc.sync.dma_start(out=mq_t, in_=mkq.ap())
        dma_x(2)
        dma_x(3)
        wq_sb = [wq_t[:, kc, :] for kc in range(KC)]
        wk_sb = [wk_t[:, kc, :] for kc in range(KC)]
        wv_sb = [wv_t[:, kc, :] for kc in range(KC)]
        wo_sb = [wo_t[:, 0, :], wo_t[:, 1, :]]

        qt_sb = [p_qt.tile([128, L], BF16, tag="qt", name=f"qt{i}") for i in range(NT)]
        kt_sb = [p_kt.tile([128, L], BF16, tag="kt", name=f"kt{i}") for i in range(NT)]
        yt_sb = [p_yt.tile([128, L], BF16, tag="yt", name=f"yt{i}") for i in range(NT)]
        v_sb = [p_v.tile([128, HPC, 65], BF16, tag="vaug", name=f"vaug{i}")
                for i in range(LT)]
        for lt in range(LT):
            nc.vector.memset(v_sb[lt][:, :, 64:65], 1.0)

        # ---- projection pieces (matmuls now, psum evac next group) ------
        # Q^T/K^T chunk: 8 accumulating matmuls, then evac via scalar copy
        # (bf16) + rotate-pairs RoPE as all-SBUF bf16 DVE ops (2x mode).
        # srot rows r hold +sin[r%32] (r%64<32) / -sin[r%32].
        def qk_piece(w_list, trg, nt, lc):
            sl = slice(lc * 512, (lc + 1) * 512)
            cell = []

            def mm():
                ps = p_mm.tile([128, 512], F32, tag="mm", name=f"psqk{nt}_{lc}")
                cell.append(ps)
                for kc in range(KC):
                    nc.tensor.matmul(
                        ps[:, :], w_list[kc][:, nt * 128:(nt + 1) * 128],
                        x_sb[lc][:, kc, :], start=(kc == 0), stop=(kc == KC - 1))

            def evac():
                ps = cell[0]
                qraw = p_ev.tile([128, 512], BF16, tag="qraw")
                nc.scalar.copy(qraw[:, :], ps[:, :])
                tmp = p_ev.tile([128, 512], BF16, tag="tmp")
                for hh in range(2):
                    b0 = hh * 64
                    nc.vector.tensor_mul(tmp[b0:b0 + 32, :], qraw[b0 + 32:b0 + 64, :],
                                         srot_t[b0 + 32:b0 + 64, sl])
                    nc.vector.tensor_mul(tmp[b0 + 32:b0 + 64, :], qraw[b0:b0 + 32, :],
                                         srot_t[b0:b0 + 32, sl])
                nc.vector.tensor_mul(trg[:, sl], qraw[:, :], cos_t[:, sl])
                nc.vector.tensor_add(trg[:, sl], trg[:, sl], tmp[:, :])

            return mm, evac

        # V chunk (row-major, one 128-row l-tile): x^T tile stationary
        def v_piece(lt):
            lc, r = lt // 4, lt % 4
            cell = []

            def mm():
                ps = p_mm.tile([128, 512], F32, tag="mm", name=f"psv{lt}")
                cell.append(ps)
                for kc in range(KC):
                    nc.tensor.matmul(
                        ps[:, 0:256], x_sb[lc][:, kc, r * 128:(r + 1) * 128],
                        wv_sb[kc][:, :], start=(kc == 0), stop=(kc == KC - 1))

            def evac():
                nc.vector.tensor_copy(
                    v_sb[lt][:, :, 0:64],
                    cell[0][:, 0:256].rearrange("p (h v) -> p h v", h=HPC))

            return mm, evac

        # one Wo output tile for chunk c, split into two pieces so the two
        # accumulating matmuls have other tensor work between them
        def wo_pieces(c, ot, cast_scalar=False, ps_fn=None):
            csl = slice(c * 512, (c + 1) * 512)
            cell = []

            def mm_a():
                if ps_fn is not None:
                    ps = ps_fn()
                else:
                    ps = p_mm.tile([128, 512], F32, tag="mm", name=f"pso{c}_{ot}")
                cell.append(ps)
                nc.tensor.matmul(ps[:, :], wo_sb[0][:, ot * 128:(ot + 1) * 128],
                                 yt_sb[0][:, csl], start=True, stop=False)

            def mm_b():
                nc.tensor.matmul(cell[0][:, :],
                                 wo_sb[1][:, ot * 128:(ot + 1) * 128],
                                 yt_sb[1][:, csl], start=False, stop=True)

            def evac():
                oc = p_oc.tile([128, 512], BF16, tag="oc")
                if cast_scalar:
                    nc.scalar.copy(oc[:, :], cell[0][:, :])
                else:
                    nc.vector.tensor_copy(oc[:, :], cell[0][:, :])
                nc.sync.dma_start(
                    out=outT.ap()[ot * 128:(ot + 1) * 128, csl], in_=oc[:, :])

            def noop():
                pass

            return [(mm_a, noop), (mm_b, evac)]

        # ---- attention for (chunk c, head-pair nt), pieces woven in -----
        # Returns softmax-normalization thunks for the caller to weave into
        # the NEXT (c, nt) iteration (keeps tensor fed at boundaries).
        def emit_attn(c, nt, norm_pieces, pieces):
            jmax = 4 * c + 3 if causal else LT - 1
            ng = jmax + 1
            csl = slice(c * 512, (c + 1) * 512)
            oaugs = [p_oa.tile([65, 512], F32, tag="oaug", name=f"oa{c}{nt}{i}")
                     for i in range(2)]
            # previous iteration's normalization: all of it must land before
            # this iteration's first O write (group 1) reuses its oaug bufs
            mm_at, ev_at = {}, {}
            for idx, (_, stage) in enumerate(norm_pieces):
                ev_at.setdefault(0 if idx < 6 else 1, []).append(stage)
            # schedule piece matmuls evenly across groups 2..; evac one later
            g0 = 2 if norm_pieces else 0
            span = max(1, ng - g0)
            for idx, pc in enumerate(pieces):
                g = min(ng - 1, g0 + (idx * span) // max(1, len(pieces)))
                if pc[0] is not None:
                    mm_at.setdefault(g, []).append(pc[0])
                ev_at.setdefault(min(ng - 1, g + 1), []).append(pc[1])

            def emit_o(j, pt, t, hh):
                nc.tensor.matmul(
                    oaugs[hh][:, t:512], v_sb[j][:, nt * 2 + hh, :],
                    pt[:, hh * 512 + t:(hh + 1) * 512],
                    start=(j == 0), stop=(j == jmax))

            def emit_s(j, st, t, hh):
                r0 = hh * 64
                nc.tensor.matmul(
                    st[:, hh * 512 + t:(hh + 1) * 512],
                    kt_sb[nt][r0:r0 + 64, j * 128:(j + 1) * 128],
                    qt_sb[nt][r0:r0 + 64, c * 512 + t:(c + 1) * 512],
                    start=True, stop=True)

            # S and O matmuls interleaved one-at-a-time: adjacent matmuls at
            # different PE tile_positions share the moving-operand port and
            # slow each other down, so never emit the two heads back-to-back
            pend = None
            for j in range(ng):
                k = j - 4 * c
                t = 128 * k if (causal and k >= 0) else 0
                st = p_st.tile([128, 1024], F32, tag="st")
                emit_s(j, st, t, 0)
                if pend is not None:
                    emit_o(pend[0], pend[1], pend[2], 0)
                emit_s(j, st, t, 1)
                for fn in mm_at.get(j, ()):
                    fn()
                pt = p_pt.tile([128, 1024], BF16, tag="pt")
                nc.scalar.activation(pt[:, t:], st[:, t:], EXP)
                if causal and k >= 0:
                    sk = 512 - 128 * k
                    nc.vector.tensor_mul(pt[:, t:], pt[:, t:],
                                         mq_t[:, sk + t:sk + 1024])
                for fn in ev_at.get(j, ()):
                    fn()
                if pend is not None:
                    emit_o(pend[0], pend[1], pend[2], 1)
                pend = (j, pt, t)
            emit_o(pend[0], pend[1], pend[2], 0)
            emit_o(pend[0], pend[1], pend[2], 1)

            # staged normalization thunks: zs copies, recips, partition
            # broadcasts (gpsimd), final scaled writes into Y^T
            zss, zrs, zbs = [], [], []

            def mk_zs(hh):
                def f():
                    zs = p_z.tile([1, 512], F32, tag="zs", name=f"zs{c}{nt}{hh}")
                    nc.vector.tensor_copy(zs[0:1, :], oaugs[hh][64:65, :])
                    zss.append(zs)
                return f

            def mk_recip(hh):
                def f():
                    zrow = p_z.tile([1, 512], F32, tag="zrow",
                                    name=f"zr{c}{nt}{hh}")
                    nc.vector.reciprocal_approx_fast(zrow[0:1, :], zss[hh][0:1, :])
                    zrs.append(zrow)
                return f

            def mk_bc(hh):
                def f():
                    zb = p_z.tile([64, 512], F32, tag="zb", name=f"zb{c}{nt}{hh}")
                    nc.gpsimd.partition_broadcast(zb[:, :], zrs[hh][0:1, :])
                    zbs.append(zb)
                return f

            def mk_mul(hh):
                def f():
                    nc.vector.tensor_mul(yt_sb[nt][hh * 64:(hh + 1) * 64, csl],
                                         oaugs[hh][0:64, :], zbs[hh][:, :])
                return f

            return [(None, mk_zs(0)), (None, mk_zs(1)),
                    (None, mk_recip(0)), (None, mk_recip(1)),
                    (None, mk_bc(0)), (None, mk_bc(1)),
                    (None, mk_mul(0)), (None, mk_mul(1))]

        # ---- main pipeline ---------------------------------------------
        def proj_half(lc, nt):
            if nt == 0:
                return [qk_piece(wq_sb, qt_sb[0], 0, lc),
                        qk_piece(wk_sb, kt_sb[0], 0, lc),
                        v_piece(lc * 4), v_piece(lc * 4 + 1)]
            return [v_piece(lc * 4 + 2), v_piece(lc * 4 + 3),
                    qk_piece(wq_sb, qt_sb[1], 1, lc),
                    qk_piece(wk_sb, kt_sb[1], 1, lc)]

        # chunk 0: only what attn(0,0) needs up front; nt1's projections are
        # woven into attn(0,0) so exp starts as early as possible
        upfront = proj_half(0, 0) + [v_piece(2), v_piece(3)]
        for mm, evac in upfront:
            mm()
            evac()
        nt1_qk0 = [qk_piece(wq_sb, qt_sb[1], 1, 0),
                   qk_piece(wk_sb, kt_sb[1], 1, 0)]

        # Wo pieces for chunk c can run any time after its normalization (yt
        # columns are never overwritten), so push them late to balance the
        # pipeline: early iterations are projection-heavy, late ones would
        # otherwise be exp-paced with an idle tensor engine.
        carry = []  # norm thunks from the previous (c, nt) iteration
        for c in range(LC):
            for nt in range(2):
                pieces = []
                if c == 0 and nt == 0:
                    pieces += nt1_qk0
                if c < LC - 1:
                    pieces += proj_half(c + 1, nt)
                ots = range(4) if nt == 0 else range(4, 8)
                if c == 2:
                    for ot in ots:
                        pieces += wo_pieces(0, ot)
                elif c == 3:
                    for cc in (1, 2):
                        for ot in ots:
                            pieces += wo_pieces(cc, ot)
                carry = emit_attn(c, nt, carry, pieces)
        # tail Wo: the kc2=0 matmuls depend only on nt0's (already finished)
        # normalization, so emit them BEFORE the final norm chain and borrow
        # the now-idle st pool for psum so six can be outstanding
        st_halves = []

        def st_ps():
            if not st_halves:
                stt = p_st.tile([128, 1024], F32, tag="st", name="wost")
                st_halves.extend([stt[:, 0:512], stt[:, 512:1024]])
            return st_halves.pop(0)

        ws = [wo_pieces(LC - 1, ot, cast_scalar=(ot % 2 == 1),
                        ps_fn=(st_ps if ot < 4 else None))
              for ot in range(8)]
        for ot in range(4):
            ws[ot][0][0]()          # a0..a3 on borrowed st psum
        ws[4][0][0]()               # a4, a5 on p_mm slots
        ws[5][0][0]()
        for _, stage in carry:      # final normalization drains over the a's
            stage()
        for x in (ws[0][1][0], ws[0][1][1], ws[1][1][0], ws[1][1][1],
                  ws[4][1][0], ws[4][1][1], ws[6][0][0],   # a6 after ev4
                  ws[2][1][0], ws[2][1][1],
                  ws[5][1][0], ws[5][1][1], ws[7][0][0],   # a7 after ev5
                  ws[3][1][0], ws[3][1][1],
                  ws[6][1][0], ws[6][1][1], ws[7][1][0], ws[7][1][1]):
            x()

    nc.compile()
    return nc


def _get_nc(causal: bool):
    key = "causal" if causal else "dense"
    if key not in _cache:
        _cache[key] = _build_nc(causal)
    return _cache[key]


def _rope_np(x):
    d, s = x.shape[-1], x.shape[-2]
    ts = np.arange(0, d, 2, dtype=np.float32)
    inv = 10000.0 ** (-ts / d)
    grid = np.arange(s, dtype=np.float32)[:, None] * inv[None, :]
    sin = np.repeat(np.sin(grid), 2, axis=-1)
    cos = np.repeat(np.cos(grid), 2, axis=-1)
    x1, x2 = x[..., ::2], x[..., 1::2]
    xs = np.stack([-x2, x1], axis=-1).reshape(x.shape)
    return x * cos + xs * sin


def _reference_np(x, mask, Wq, Wk, Wv, Wo):
    b, l, d = x.shape
    h, k_sz = H, D // H
    split = lambda t: t.reshape(b, l, h, k_sz).transpose(0, 2, 1, 3)
    q = split((x @ Wq) / np.sqrt(np.float32(d)))
    q = _rope_np(q)
    k = _rope_np(split(x @ Wk))
    v = split(x @ Wv)
    logits = np.einsum("bhik,bhjk->bhij", q, k) + mask
    m = logits.max(axis=-1, keepdims=True)
    p = np.exp(logits - m)
    a = p / p.sum(axis=-1, keepdims=True)
    y = np.einsum("bhij,bhjv->bhiv", a, v)
    y = y.transpose(0, 2, 1, 3).reshape(b, l, d)
    return (y @ Wo).astype(np.float32)


def _spectral_norm(w, iters=12):
    rng = np.random.default_rng(0)
    v = rng.standard_normal(w.shape[1]).astype(np.float32)
    for _ in range(iters):
        u = w @ v
        u /= (np.linalg.norm(u) + 1e-30)
        v = w.T @ u
        nv = np.linalg.norm(v)
        v /= (nv + 1e-30)
    return float(nv)


def _host_consts():
    inv = 10000.0 ** (-np.arange(0, HD, 2, dtype=np.float32) / HD)
    grid = np.arange(L, dtype=np.float32)[None, :] * inv[:, None]   # [32, L]
    cos32 = np.cos(grid).astype(np.float32)
    sin32 = np.sin(grid).astype(np.float32)
    cos128 = np.ascontiguousarray(np.tile(cos32, (4, 1)))
    # srot rows r: +sin[r%32] for r%64 < 32, -sin[r%32] otherwise
    srot128 = np.ascontiguousarray(
        np.tile(np.concatenate([sin32, -sin32], axis=0), (2, 1)))
    # cyclic causal mask: mkq[p, u] = T0[u mod 512] with
    # T0[p, t] = (t >= p) for t < 128, else 1
    t0 = np.ones((128, 512), dtype=np.float32)
    t0[:, 0:128] = (np.arange(128)[None, :] >= np.arange(128)[:, None])
    mkq = np.ascontiguousarray(np.concatenate([t0, t0, t0], axis=1))
    return cos128, srot128, mkq


def _make_in_maps(x, Wq, Wk, Wv, Wo):
    import ml_dtypes
    bf16 = ml_dtypes.bfloat16

    cos128, srot128, mkq = _host_consts()
    cos128 = cos128.astype(bf16)
    srot128 = srot128.astype(bf16)
    mkq = mkq.astype(bf16)
    perm = np.concatenate([np.arange(0, 64, 2), np.arange(1, 64, 2)])
    Wq_s = (Wq / np.sqrt(np.float32(D))).astype(np.float32)
    in_maps = []
    for core in range(NCORES):
        bi, g = core // 4, core % 4
        xT_b = np.ascontiguousarray(x[bi].T.astype(bf16))
        wq_c = np.empty((D, 256), np.float32)
        wk_c = np.empty((D, 256), np.float32)
        for hh in range(HPC):
            h_abs = g * HPC + hh
            wq_c[:, hh * 64:(hh + 1) * 64] = Wq_s[:, h_abs * 64:(h_abs + 1) * 64][:, perm]
            wk_c[:, hh * 64:(hh + 1) * 64] = Wk[:, h_abs * 64:(h_abs + 1) * 64][:, perm]
        in_maps.append({
            "xT": xT_b,
            "wq": wq_c.astype(bf16),
            "wk": wk_c.astype(bf16),
            "wv": np.ascontiguousarray(Wv[:, g * 256:(g + 1) * 256].astype(bf16)),
            "wo": np.ascontiguousarray(Wo[g * 256:(g + 1) * 256, :].astype(bf16)),
            "cos128": cos128, "srot128": srot128, "mkq": mkq,
        })
    return in_maps


def kernel(x, mask, Wq, Wk, Wv, Wo):
    from concourse.bass_utils import run_bass_kernel_spmd

    x = np.asarray(x, dtype=np.float32)
    mask = np.asarray(mask, dtype=np.float32)
    Wq = np.asarray(Wq, dtype=np.float32)
    Wk = np.asarray(Wk, dtype=np.float32)
    Wv = np.asarray(Wv, dtype=np.float32)
    Wo = np.asarray(Wo, dtype=np.float32)

    # classify the mask
    m = mask.reshape(L, L)
    tril = np.tril(np.ones((L, L), dtype=bool))
    visible = m > -1e6
    if np.array_equal(visible, tril) and not m[tril].any():
        causal = True
    elif not m.any():
        causal = False
    else:
        return _reference_np(x, mask, Wq, Wk, Wv, Wo)

    # overflow guard for the no-max-subtraction softmax
    xr = float(np.sqrt((x * x).sum(axis=2).max()))
    bound = (xr * _spectral_norm(Wq) / np.sqrt(D)) * (xr * _spectral_norm(Wk))
    if bound > 60.0:
        return _reference_np(x, mask, Wq, Wk, Wv, Wo)

    in_maps = _make_in_maps(x, Wq, Wk, Wv, Wo)
    nc = _get_nc(causal)
    res = run_bass_kernel_spmd(nc, in_maps, core_ids=list(range(NCORES)))

    out = np.empty((B, L, D), dtype=np.float32)
    for bi in range(B):
        acc = res.results[bi * 4]["outT"].astype(np.float32)
        for g in range(1, 4):
            acc += res.results[bi * 4 + g]["outT"].astype(np.float32)
        out[bi] = acc.T
    return out
